# revision 15
# baseline (speedup 1.0000x reference)
"""Cross-attention (B=4, N=2048, C=768, H=12, HD=64) on 8 TRN2 NeuronCores.

Sharding: core = (batch, head_group), 4 batches x 2 groups of 6 heads.
Each core computes its group's Q/K/V projections, per-head-dim LayerNorm,
attention, and a partial output projection; the host sums the two group
partials per batch and adds the bias.

Key optimizations over the f32r baseline:
 - The attn_mask masks whole QUERY rows; a masked query's output is exactly
   mean(v) @ Wp + bp, computed on the host in numpy.  The device only sees
   the gathered unmasked queries (NQ ~= 1060 of 2048), cutting S/PV/exp/
   q-proj/out-proj work by ~2x combined with the next point.
 - All matmuls run in fp16: 1 column/cycle streaming (fp32 runs HIGH/LOW
   two-pass) and FastWeightLoad hides LDWEIGHTS (disabled for fp32).
 - S = k~.T @ q~ contracts over HD=64 only, so head pairs are packed into
   the 128-row PE array with tile_position row tiling (auto-derived from
   base_partition 0/64) and run concurrently: S cost halves.
 - LN mean/meansq matmuls are column-tiled (out partitions 0-5 / 32-37 of
   one PSUM tile), the rs/murs broadcast matmuls are row-tiled (partitions
   0-5 / 32-37) - each pair runs concurrently.
 - Softmax needs no row max: LN bounds |S| <= 8, so exp(S) in [3e-4, 3e3],
   safely inside fp16/fp32 range.  The denominator rides along as a ones
   column appended to v (PV out row 64).  O is scaled by 2^-6 before the
   fp16 copy to dodge overflow; the reciprocal uses the same scaled den.
PSUM budget (8 banks): spA+spB (2+2) + poA+poB (1+1) + mix pp (2).
"""

import math

import numpy as np

import concourse.bass as bass
import concourse.mybir as mybir
from concourse import tile
from concourse import bass_utils
from concourse.tile_scheduler import N_PROCS
from concourse.vector_clock import ScopedClock, VectorClock

F32 = mybir.dt.float32
F16 = mybir.dt.float16
AF = mybir.ActivationFunctionType
OP = mybir.AluOpType

B, N, C, H, HD = 4, 2048, 768, 12, 64
G = 2                 # head groups (tensor parallel)
HPG = H // G          # 6 heads per group
CL = HPG * HD         # 384 local channels
P = 128
NT = CL // P          # 3 tiles of local channels
CT = C // P           # 6 contraction tiles
TT = N // P           # 16 key-token tiles
KCH = 512             # kv chunk size
EPS = 1e-5
SCALE = HD ** -0.5
LNB = -0.5 * math.log(HD)   # ln(SCALE): folded into the q-LN exp bias
OSHIFT = 2.0 ** -6          # pre-normalize scale to keep |o| in fp16 range
NCORES = 8

_nop_ctr = [0]


class _FixedTileContext(tile.TileContext):
    """Workaround for a walrus build that allows at most ONE sync-wait per
    instruction: split multi-wait instructions into single-wait NoOps on the
    same engine, and emit the kernel-tail drain's waits as a nop chain."""

    def _split_multiwait(self, insts):
        out = []
        for inst in insts:
            si = getattr(inst, "sync_info", None)
            waits = list(si.on_wait) if si is not None and si.on_wait else []
            if len(waits) > 1:
                eng = inst.engine
                for w in waits[:-1]:
                    _nop_ctr[0] += 1
                    nop = mybir.InstNoOp(
                        name=f"I-waitsplit-{_nop_ctr[0]}", ins=[], outs=[]
                    )
                    nop.engine = eng
                    nop.sync_info = mybir.SyncInfo(on_wait=[w], on_update=[])
                    self.nc.register_instruction(nop)
                    out.append(nop)
                inst.sync_info = mybir.SyncInfo(
                    on_wait=[waits[-1]], on_update=list(si.on_update)
                )
            out.append(inst)
        return out

    def _lower_ordered_insts(self, ordered):
        ordered = {bb: self._split_multiwait(ins) for bb, ins in ordered.items()}
        super()._lower_ordered_insts(ordered)

    def _drain_and_barrier(self, tick_clock, wait_clock):
        gc = tick_clock.global_clock
        vals = [gc[p] for p in range(N_PROCS)]
        for p in [q for q, v in enumerate(vals) if v > 0]:
            partial = VectorClock(
                [vals[q] if q == p else 0 for q in range(N_PROCS)]
            )
            nop = self.nc.sync.nop(nofuse=True, hint="tail_drain_wait")
            wait_clock.add_sem_waits(nop.ins, ScopedClock({None: partial}))
        self.nc.sync.drain()
        self.nc.all_engine_barrier()
        assert self.sems is not None
        popped = self.nc._tile_sem_poison_stack.pop()
        assert popped is self._sem_poison
        self.nc.clear_and_free_semaphores(list(self.sems.allocated().values()))
        self.nc.all_engine_barrier()


def _mm(nc, out, lhsT, rhs, start, stop):
    nc.tensor.matmul(
        out, lhsT, rhs, start=start, stop=stop, skip_group_check=True
    )


def _chunks(total, size=512):
    out, o = [], 0
    while o < total:
        s = min(size, total - o)
        out.append((o, s))
        o += s
    return out


def _step(fill):
    """Advance the round-robin fill queue by one piece (if any)."""
    while fill:
        gen = fill[0]
        try:
            next(gen)
            fill.rotate(-1)
            return
        except StopIteration:
            fill.popleft()


def _lane_step(fifo, fill):
    """One piece from the serial fifo lane (head only -- generators that
    share tile tags deadlock if interleaved) or the rotating fill lane.
    Returns the piece's yield value (True = PE work) or None if empty."""
    while fifo:
        try:
            return next(fifo[0])
        except StopIteration:
            fifo.popleft()
    while fill:
        gen = fill[0]
        try:
            v = next(gen)
            fill.rotate(-1)
            return v
        except StopIteration:
            fill.popleft()
    return None


def _step2(fifo, fill, warm):
    """Issue fill pieces until one carries PE work (max 3), falling back
    to keep-warm matmuls so the HAM clock gate never sees an idle PE."""
    for _ in range(3):
        v = _lane_step(fifo, fill)
        if v:
            return
        if v is None:
            break
    _step(warm)


def _drain(gen):
    for _ in gen:
        pass


def _body(tc, aps, NQ):
    import collections
    nc = tc.nc
    qxT, kvxT, wq, wk, wv, wp, colsel, bcast, vones, ones16, outT = aps
    qch = _chunks(NQ)
    kch = _chunks(N)

    cpool = tc.alloc_tile_pool(name="consts", bufs=1)
    bpool = tc.alloc_tile_pool(name="big", bufs=1)

    colsel_sb = cpool.tile([P, NT, HPG], F16, name="colsel", tag="colsel")
    nc.sync.dma_start(colsel_sb[:], colsel[:])
    bcast_sb = cpool.tile([38, NT, P], F16, name="bcast", tag="bcast")
    nc.sync.dma_start(bcast_sb[:], bcast[:])
    ones16_sb = cpool.tile([1, HD], F16, name="ones16", tag="ones16")
    nc.sync.dma_start(ones16_sb[:], ones16[:])
    eps_sb = cpool.tile([HPG, 1], F32, name="eps", tag="eps")
    nc.vector.memset(eps_sb[:], EPS)
    lnb_sb = cpool.tile([HPG, 1], F32, name="lnb", tag="lnb")
    nc.vector.memset(lnb_sb[:], LNB)

    q_sb = [bpool.tile([P, NQ], F16, name=f"q{t}", tag=f"q{t}") for t in range(NT)]
    k_sb = [bpool.tile([P, N], F16, name=f"k{t}", tag=f"k{t}") for t in range(NT)]
    v_sb = bpool.tile([P, TT, HPG, HD + 1], F16, name="v", tag="v")

    # DMA order matters for the cold start: wk + kv0's x tiles first so
    # the first k-proj matmul can issue ASAP; wp (tails only) last.
    wk_sb = bpool.tile([P, CT, CL], F16, name="wk", tag="wk")
    wv_sb = bpool.tile([P, CT, CL], F16, name="wv", tag="wv")
    wq_sb = bpool.tile([P, CT, CL], F16, name="wq", tag="wq")
    nc.sync.dma_start(wk_sb[:], wk.rearrange("(ct p) m -> p ct m", p=P))
    wp_sb = bpool.tile([P, NT, C], F16, name="wp", tag="wp")

    # PSUM pools: spA(2) + spB(2) + poA(1) + poB(1) + pp(2x1) = 8 banks
    ps_sp = tc.alloc_tile_pool(name="ps_sp", bufs=1, space="PSUM")
    ps_po = tc.alloc_tile_pool(name="ps_po", bufs=1, space="PSUM")
    ps_mix = tc.alloc_tile_pool(name="ps_mix", bufs=2, space="PSUM")

    x_pool = tc.alloc_tile_pool(name="x", bufs=6)
    sq_pool = tc.alloc_tile_pool(name="sq", bufs=2)
    st_pool = tc.alloc_tile_pool(name="st", bufs=3)
    e_pool = tc.alloc_tile_pool(name="e", bufs=2)
    o_pool = tc.alloc_tile_pool(name="o", bufs=2)
    den_pool = tc.alloc_tile_pool(name="den", bufs=2)
    out_pool = tc.alloc_tile_pool(name="ot", bufs=3)

    nq = len(qch)
    NKB = len(kch)                       # 4 k-blocks of 4 k-tiles each
    KPB = KCH // P                       # 4 k-tiles per block
    # flash accumulators: [65, ch] fp32 per (q-chunk, tile, head)
    o_acc = {}
    for qi, (qo, ch) in enumerate(qch):
        for t in range(NT):
            for hh in range(2):
                o_acc[(qi, t, hh)] = bpool.tile(
                    [HD + 1, ch], F32, name=f"oa{qi}_{t}_{hh}",
                    tag=f"oa{qi}_{t}_{hh}")

    def ln_gen(dst, cs, ch, is_q):
        """Projection LN tail: mean/meansq (column-tiled), rs/murs, apply
        (row-tiled broadcast matmuls).  All PSUM via the 'pp' tag."""
        mums = ps_mix.tile([38, ch], F32, name="mums", tag="pp")
        for t in range(NT):
            sq = sq_pool.tile([P, ch], F16, name="sq", tag="sq")
            nc.vector.tensor_tensor(sq[:], dst[t][:, cs], dst[t][:, cs],
                                    OP.mult)
            _mm(nc, mums[0:HPG, :], colsel_sb[:, t, :], dst[t][:, cs],
                t == 0, t == NT - 1)
            _mm(nc, mums[32:38, :], colsel_sb[:, t, :], sq[:],
                t == 0, t == NT - 1)
        yield True
        stf = st_pool.tile([HPG, 3 * ch], F32, name="stf", tag="stf")
        sth = st_pool.tile([38, ch], F16, name="sth", tag="sth")
        mu = stf[:, 0:ch]
        work = stf[:, ch:2 * ch]
        lnv = stf[:, 2 * ch:3 * ch]
        nc.vector.tensor_copy(mu, mums[0:HPG, :])
        nc.vector.scalar_tensor_tensor(work, mu, 1.0, mu, OP.mult, OP.mult)
        nc.vector.tensor_tensor(work, mums[32:38, :], work, OP.subtract)
        nc.scalar.activation(lnv, work, AF.Ln, bias=eps_sb[:])
        # rs = exp(-0.5*ln(var+eps) [+ ln(scale) for q]) ; murs = -mu*rs
        nc.scalar.activation(sth[0:HPG, :], lnv, AF.Exp, scale=-0.5,
                             bias=(lnb_sb[:] if is_q else 0.0))
        nc.vector.scalar_tensor_tensor(sth[32:38, :], mu, -1.0,
                                       sth[0:HPG, :], OP.mult, OP.mult)
        yield False
        for t in range(NT):
            rr = ps_mix.tile([P, ch], F32, name="rr", tag="pp")
            _mm(nc, rr[:], bcast_sb[0:HPG, t, :], sth[0:HPG, :], True, True)
            mr = ps_mix.tile([P, ch], F32, name="mr", tag="pp")
            _mm(nc, mr[:], bcast_sb[32:38, t, :], sth[32:38, :], True, True)
            nc.vector.tensor_tensor(dst[t][:, cs], dst[t][:, cs], rr[:],
                                    OP.mult)
            nc.vector.tensor_tensor(dst[t][:, cs], dst[t][:, cs], mr[:],
                                    OP.add)
            yield True

    def kv_gen(c):
        co, ch = kch[c]
        cs = slice(co, co + ch)
        xts = []
        for ct in range(CT):
            xt = x_pool.tile([P, ch], F16, name="xt", tag=f"xk{c}")
            nc.sync.dma_start(xt[:], kvxT[ct * P:(ct + 1) * P, cs])
            xts.append(xt)
        yield False
        for t in range(NT):
            pp = ps_mix.tile([P, ch], F32, name="pp", tag="pp")
            for ct in range(CT):
                _mm(nc, pp[:], wk_sb[:, ct, t * P:(t + 1) * P], xts[ct][:],
                    ct == 0, ct == CT - 1)
            nc.vector.tensor_copy(k_sb[t][:, cs], pp[:])
            yield True
        for tl in range(ch // P):
            ttk = co // P + tl
            vp = ps_mix.tile([P, CL], F32, name="vp", tag="pp")
            for ct in range(CT):
                _mm(nc, vp[:], xts[ct][:, tl * P:(tl + 1) * P],
                    wv_sb[:, ct, :], ct == 0, ct == CT - 1)
            nc.vector.tensor_copy(
                v_sb[:, ttk, :, 0:HD],
                vp[:].rearrange("p (h d) -> p h d", h=HPG))
            yield True
        yield from ln_gen(k_sb, cs, ch, False)

    def q_gen(i):
        qo, ch = qch[i]
        cs = slice(qo, qo + ch)
        xts = []
        for ct in range(CT):
            xt = x_pool.tile([P, ch], F16, name="xt", tag=f"xq{i}")
            nc.sync.dma_start(xt[:], qxT[ct * P:(ct + 1) * P, cs])
            xts.append(xt)
        yield False
        for t in range(NT):
            pp = ps_mix.tile([P, ch], F32, name="pp", tag="pp")
            for ct in range(CT):
                _mm(nc, pp[:], wq_sb[:, ct, t * P:(t + 1) * P], xts[ct][:],
                    ct == 0, ct == CT - 1)
            nc.vector.tensor_copy(q_sb[t][:, cs], pp[:])
            yield True
        yield from ln_gen(q_sb, cs, ch, True)

    def warm_gen(nmm):
        """Keep-warm matmuls: the HAM clock gate demotes the PE to 1.2 GHz
        when a ~3.4us window falls below ~85%% busy; these fill otherwise
        idle slots in exp-bound stretches at trivial cost."""
        for _ in range(nmm):
            wmp = ps_mix.tile([HD, KCH], F32, name="wmp", tag="pp")
            _mm(nc, wmp[:], ones16_sb[:], k_sb[0][0:1, 0:KCH], True, True)
            yield True

    def attn_unit(qi, t, blk, fill, fifo, warm):
        """S + exp + PV for one (q-chunk, head-pair) over one k-block
        (4 k-tiles), accumulating into o_acc via DVE."""
        qo, ch = qch[qi]
        qs = slice(qo, qo + ch)
        hA, hB = 2 * t, 2 * t + 1
        poA = ps_po.tile([HD + 1, ch], F32, name="poA", tag="poA")
        poB = ps_po.tile([HD + 1, ch], F32, name="poB", tag="poB")
        for g in range(KPB // 2):
            spA = ps_sp.tile([P, 2 * ch], F32, name="spA", tag="spA")
            spB = ps_sp.tile([P, 2 * ch], F32, name="spB", tag="spB")
            for j in range(2):
                kt = blk * KPB + 2 * g + j
                # row-tiled pair: head A in PE rows 0-63, head B in 64-127
                _mm(nc, spA[:, j * ch:(j + 1) * ch],
                    k_sb[t][0:HD, kt * P:(kt + 1) * P],
                    q_sb[t][0:HD, qs], True, True)
                _mm(nc, spB[:, j * ch:(j + 1) * ch],
                    k_sb[t][HD:P, kt * P:(kt + 1) * P],
                    q_sb[t][HD:P, qs], True, True)
            eA = e_pool.tile([P, 2 * ch], F16, name="eA", tag="eA")
            eB = e_pool.tile([P, 2 * ch], F16, name="eB", tag="eB")
            nc.scalar.activation(eA[:], spA[:], AF.Exp)
            nc.scalar.activation(eB[:], spB[:], AF.Exp)
            for j in range(2):
                lk = 2 * g + j
                kt = blk * KPB + lk
                _mm(nc, poA[:], v_sb[:, kt, hA, :],
                    eA[:, j * ch:(j + 1) * ch], lk == 0, lk == KPB - 1)
                _mm(nc, poB[:], v_sb[:, kt, hB, :],
                    eB[:, j * ch:(j + 1) * ch], lk == 0, lk == KPB - 1)
            _step2(fifo, fill, warm)
        accA, accB = o_acc[(qi, t, 0)], o_acc[(qi, t, 1)]
        if blk == 0:
            nc.vector.tensor_copy(accA[:], poA[:])
            nc.vector.tensor_copy(accB[:], poB[:])
        else:
            nc.vector.tensor_tensor(accA[:], accA[:], poA[:], OP.add)
            nc.vector.tensor_tensor(accB[:], accB[:], poB[:], OP.add)

    def tail_gen(qi):
        """Normalize by the softmax denominator and project out."""
        qo, ch = qch[qi]
        qs = slice(qo, qo + ch)
        den = den_pool.tile([1, HPG * ch], F32, name="den", tag="den")
        for t in range(NT):
            for hh in range(2):
                h = 2 * t + hh
                nc.vector.tensor_scalar_mul(
                    den[0:1, h * ch:(h + 1) * ch],
                    o_acc[(qi, t, hh)][HD:HD + 1, :], OSHIFT)
        # batched reciprocal: repack [1, 6*ch] -> [32, 6*ch/32] (DVE
        # reciprocal cost scales with free size only), invert, unpack fp16
        w32 = HPG * ch // 32
        dpk = den_pool.tile([32, w32], F32, name="dpk", tag="dpk")
        nc.sync.dma_start(dpk[:], den[0:1, :])
        rpk = den_pool.tile([32, w32], F32, name="rpk", tag="rpk")
        nc.vector.reciprocal(rpk[:], dpk[:])
        rpk16 = den_pool.tile([32, w32], F16, name="rpk16", tag="rpk16")
        nc.vector.tensor_copy(rpk16[:], rpk[:])
        denr = den_pool.tile([1, HPG * ch], F16, name="denr", tag="denr")
        nc.sync.dma_start(denr[0:1, :], rpk16[:])
        yield False
        o_t = [o_pool.tile([P, ch], F16, name=f"o{t}", tag=f"o{t}")
               for t in range(NT)]
        for t in range(NT):
            for hh in range(2):
                h = 2 * t + hh
                rb = ps_mix.tile([HD, ch], F32, name="rb", tag="pp")
                _mm(nc, rb[:], ones16_sb[:],
                    denr[0:1, h * ch:(h + 1) * ch], True, True)
                # o = (acc * 2^-6) * (1 / (den * 2^-6)) -- fp16-safe
                nc.vector.scalar_tensor_tensor(
                    o_t[t][hh * HD:(hh + 1) * HD, :],
                    o_acc[(qi, t, hh)][0:HD, :], OSHIFT, rb[:],
                    OP.mult, OP.mult)
            yield True
        for m in range(CT):
            pp = ps_mix.tile([P, ch], F32, name="op", tag="pp")
            for t in range(NT):
                _mm(nc, pp[:], wp_sb[:, t, m * P:(m + 1) * P], o_t[t][:],
                    t == 0, t == NT - 1)
            ot = out_pool.tile([P, ch], F16, name="ot", tag="ot")
            nc.vector.tensor_copy(ot[:], pp[:])
            nc.sync.dma_start(outT[m * P:(m + 1) * P, qs], ot[:])
            yield True

    # ---- schedule ----------------------------------------------------
    # Head: kv blocks 0-1 and q chunk 0 run dense (PE-bound, ACT idle).
    # Attention (exp/ACT-bound) then runs block-major with the remaining
    # projection work fed into the PE's idle slots so the HAM clock gate
    # never sees an idle PE window: block0 <- q chunks 1+, block1 <- kv2,
    # block2 <- kv3, block3 <- per-chunk output tails.
    # Head: only kv0 + q0, interleaved round-robin so one chunk's serial
    # LN chain (DVE+ACT) overlaps the other's projection matmuls -- a
    # >3.4us PE-idle window demotes the HAM clock gate to 1.2 GHz.  All
    # remaining projection work (kv1-3, q1+) feeds the attention blocks'
    # PE idle slots: attention alone is exp/ACT-bound at ~56% PE busy,
    # and a window below ~85% busy also demotes the clock.
    kvg = {1: kv_gen(1), 2: kv_gen(2), 3: kv_gen(3)}
    qgens = {i: q_gen(i) for i in range(1, nq)}
    head = collections.deque([kv_gen(0), q_gen(0)])
    _step(head)                # kv0 x-DMA burst (right behind wk)
    nc.sync.dma_start(wq_sb[:], wq.rearrange("(ct p) m -> p ct m", p=P))
    _step(head)                # q0 x-DMA burst
    nc.sync.dma_start(wv_sb[:], wv.rearrange("(ct p) m -> p ct m", p=P))
    nc.sync.dma_start(v_sb[:, :, :, HD], vones[:])
    for gen in list(kvg.values()) + list(qgens.values()):
        next(gen, None)        # prefetch x-DMAs for later chunks too
    nc.sync.dma_start(wp_sb[:], wp.rearrange("(t p) m -> p t m", p=P))
    while head:
        _step(head)
    fill = collections.deque([kvg[1]])
    fifo = collections.deque(qgens.values())
    warm = collections.deque()
    warm.append(warm_gen(24))
    for blk in range(NKB):
        # producers must be fully issued before their consumers (the tile
        # framework orders by issue): force-drain whatever the fill queue
        # hasn't finished by the time it's needed.
        if blk == 1:
            _drain(kvg[1])
            fill.append(kvg[2])
            warm.append(warm_gen(40))
        elif blk == 2:
            _drain(kvg[2])
            fill.append(kvg[3])
            warm.append(warm_gen(40))
        elif blk == 3:
            _drain(kvg[3])
            warm.append(warm_gen(30))
        for qi in range(nq):
            if blk == 0 and qi in qgens:
                _drain(qgens[qi])
            for t in range(NT):
                attn_unit(qi, t, blk, fill, fifo, warm)
            if blk == NKB - 1:
                fifo.append(tail_gen(qi))
    while fifo or fill:
        _step2(fifo, fill, warm)

    for pool in (out_pool, den_pool, o_pool, e_pool, st_pool, sq_pool,
                 x_pool, ps_mix, ps_po, ps_sp, bpool, cpool):
        pool.release()


def build_bass(NQ):
    nc = bass.Bass(trn_type="TRN2", debug=False, num_devices=NCORES)
    qxT = nc.dram_tensor("qxT", [C, NQ], F16, kind="ExternalInput").ap()
    kvxT = nc.dram_tensor("kvxT", [C, N], F16, kind="ExternalInput").ap()
    wq = nc.dram_tensor("wq", [C, CL], F16, kind="ExternalInput").ap()
    wk = nc.dram_tensor("wk", [C, CL], F16, kind="ExternalInput").ap()
    wv = nc.dram_tensor("wv", [C, CL], F16, kind="ExternalInput").ap()
    wp = nc.dram_tensor("wp", [CL, C], F16, kind="ExternalInput").ap()
    colsel = nc.dram_tensor("colsel", [P, NT, HPG], F16,
                            kind="ExternalInput").ap()
    bcast = nc.dram_tensor("bcast", [38, NT, P], F16,
                           kind="ExternalInput").ap()
    vones = nc.dram_tensor("vones", [P, TT, HPG], F16,
                           kind="ExternalInput").ap()
    ones16 = nc.dram_tensor("ones16", [1, HD], F16,
                            kind="ExternalInput").ap()
    outT = nc.dram_tensor("outT", [C, NQ], F16, kind="ExternalOutput").ap()
    aps = (qxT, kvxT, wq, wk, wv, wp, colsel, bcast, vones, ones16, outT)
    with _FixedTileContext(nc) as tc:
        _body(tc, aps, NQ)
    return nc


def make_in_maps(q_x, kv_x, attn_mask, Wq, Wkv, Wp, NQ, idxs):
    colsel = np.zeros((P, NT, HPG), np.float16)
    bcast = np.zeros((38, NT, P), np.float16)
    for t in range(NT):
        for pp in range(P):
            h = 2 * t + pp // HD
            colsel[pp, t, h] = 1.0 / HD
            bcast[h, t, pp] = 1.0
    bcast[32:38] = bcast[0:HPG]  # mirror for the row-tiled murs broadcast
    ones16 = np.ones((1, HD), np.float16)
    vones = np.ones((P, TT, HPG), np.float16)

    in_maps = []
    for core in range(NCORES):
        b, g = core // G, core % G
        sl = slice(g * CL, (g + 1) * CL)
        idx = idxs[b]
        pad = np.zeros(NQ, np.int64)
        pad[:len(idx)] = idx
        if len(idx) < NQ:
            pad[len(idx):] = idx[0] if len(idx) else 0
        in_maps.append({
            "qxT": np.ascontiguousarray(q_x[b][pad].T.astype(np.float16)),
            "kvxT": np.ascontiguousarray(kv_x[b].T.astype(np.float16)),
            "wq": np.ascontiguousarray(Wq[sl].T.astype(np.float16)),
            "wk": np.ascontiguousarray(Wkv[sl].T.astype(np.float16)),
            "wv": np.ascontiguousarray(
                Wkv[C + g * CL:C + (g + 1) * CL].T.astype(np.float16)),
            "wp": np.ascontiguousarray(Wp[:, sl].T.astype(np.float16)),
            "colsel": colsel,
            "bcast": bcast,
            "vones": vones,
            "ones16": ones16,
        })
    return in_maps


_NC_CACHE = {}


def get_nc(NQ):
    if NQ not in _NC_CACHE:
        _NC_CACHE[NQ] = build_bass(NQ)
    return _NC_CACHE[NQ]


def prepare(q_x, kv_x, attn_mask, Wq, Wkv, Wp):
    mask = np.asarray(attn_mask).astype(bool)
    idxs = [np.flatnonzero(mask[b]) for b in range(B)]
    numax = max(1, max(len(i) for i in idxs))
    NQ = ((numax + 31) // 32) * 32
    nc = get_nc(NQ)
    in_maps = make_in_maps(q_x, kv_x, mask, Wq, Wkv, Wp, NQ, idxs)
    return nc, in_maps, idxs


def kernel(q_x, kv_x, attn_mask, Wq, Wkv, qn_w, qn_b, kn_w, kn_b, Wp, bp,
           _profile=None):
    q_x = np.asarray(q_x, np.float32)
    kv_x = np.asarray(kv_x, np.float32)
    Wq = np.asarray(Wq, np.float32)
    Wkv = np.asarray(Wkv, np.float32)
    Wp = np.asarray(Wp, np.float32)
    bp = np.asarray(bp, np.float32)
    if not (np.all(np.asarray(qn_w) == 1) and np.all(np.asarray(qn_b) == 0)
            and np.all(np.asarray(kn_w) == 1) and np.all(np.asarray(kn_b) == 0)):
        raise NotImplementedError("kernel specialized to identity q/k norms")

    nc, in_maps, idxs = prepare(q_x, kv_x, attn_mask, Wq, Wkv, Wp)
    res = bass_utils.run_bass_kernel_spmd(
        nc, in_maps, core_ids=list(range(NCORES)))
    if _profile is not None:
        _profile.append(res)

    # masked-query rows: softmax over an all -1e9 row is uniform, so the
    # output is exactly mean_k(v) @ Wp.T + bp -- pure host math.
    vmean = kv_x.mean(axis=1) @ Wkv[C:].T          # [B, C]
    ymask = vmean @ Wp.T + bp                      # [B, C]
    out = np.empty((B, N, C), np.float32)
    for b in range(B):
        acc = (res.results[G * b]["outT"].astype(np.float32)
               + res.results[G * b + 1]["outT"].astype(np.float32))
        out[b] = ymask[b]
        nb = len(idxs[b])
        out[b, idxs[b]] = acc.T[:nb] + bp
    return out
